# revision 7
# baseline (speedup 1.0000x reference)
"""Trainium2 Bass kernel for 3D attention block (GroupNorm + 1x1x1-conv QKV +
4-head attention over N=4096 + out-projection + residual).

Sharding: 8 cores = 2 batches x 4 query-slices (sequence parallel).  Each core
receives its batch's x rolled by -n0 along the flattened spatial axis, so the
SPMD program always computes queries [0:1024] of its local view; softmax /
GroupNorm / the value contraction are invariant to the roll.  No cross-core
communication is needed; the host concatenates the 8 output slices.
"""

import sys

sys.path.insert(0, "/opt/trn_rl_repo")

import numpy as np

# Problem constants (hardcoded per harness contract).
B = 2
C = 128
D3 = (16, 16, 16)
N = 4096
NH = 4
HD = 32
GROUPS = 32
EPS = 1e-5
SCALE = float(HD) ** -0.5

NCORES = 8
NSPLIT = 4          # query-slices per batch
NSLICE = N // NSPLIT  # 1024 queries per core
NT = NSLICE // 512    # 512-wide query tiles per core
MC = N // 128         # 128-wide key chunks

_nc_cache = {}


def _build():
    import concourse.bass as bass
    import concourse.tile as tile
    from concourse import bacc, mybir

    f32 = mybir.dt.float32
    bf16 = mybir.dt.bfloat16
    AF = mybir.ActivationFunctionType
    ALU = mybir.AluOpType

    nc = bacc.Bacc()

    xr_d = nc.declare_dram_parameter("xr", [C, N], f32, isOutput=False)
    qkvwT_d = nc.declare_dram_parameter("qkvwT", [C, 3 * C], f32, isOutput=False)
    w4_d = nc.declare_dram_parameter("w4", [HD, NH, C], f32, isOutput=False)
    qb_d = nc.declare_dram_parameter("qb", [C, 1], f32, isOutput=False)
    kb_d = nc.declare_dram_parameter("kb", [C, 1], f32, isOutput=False)
    vb4_d = nc.declare_dram_parameter("vb4", [HD, NH], f32, isOutput=False)
    ob_d = nc.declare_dram_parameter("ob", [C, 1], f32, isOutput=False)
    gnw_d = nc.declare_dram_parameter("gnw", [C, 1], f32, isOutput=False)
    gnb_d = nc.declare_dram_parameter("gnb", [C, 1], f32, isOutput=False)
    g2_d = nc.declare_dram_parameter("g2", [C, C], f32, isOutput=False)
    y_d = nc.declare_dram_parameter("y", [C, NSLICE], f32, isOutput=True)

    with tile.TileContext(nc) as tc:
        with tc.tile_pool(name="singles", bufs=1) as singles:
            # ---- input DMAs ----
            x_sb = singles.tile([C, N], f32)
            for t in range(8):
                nc.sync.dma_start(
                    out=x_sb[:, t * 512 : (t + 1) * 512],
                    in_=xr_d[:, t * 512 : (t + 1) * 512],
                )
            qkvwT_f = singles.tile([C, 3 * C], f32)
            nc.sync.dma_start(out=qkvwT_f, in_=qkvwT_d[:, :])
            w4_f = singles.tile([HD, NH, C], f32)
            nc.sync.dma_start(out=w4_f, in_=w4_d[:, :, :])
            qb_sb = singles.tile([C, 1], f32)
            nc.sync.dma_start(out=qb_sb, in_=qb_d[:, :])
            kb_sb = singles.tile([C, 1], f32)
            nc.sync.dma_start(out=kb_sb, in_=kb_d[:, :])
            vb4_sb = singles.tile([HD, NH], f32)
            nc.sync.dma_start(out=vb4_sb, in_=vb4_d[:, :])
            ob_sb = singles.tile([C, 1], f32)
            nc.sync.dma_start(out=ob_sb, in_=ob_d[:, :])
            gnw_sb = singles.tile([C, 1], f32)
            nc.sync.dma_start(out=gnw_sb, in_=gnw_d[:, :])
            gnb_sb = singles.tile([C, 1], f32)
            nc.sync.dma_start(out=gnb_sb, in_=gnb_d[:, :])
            g2_sb = singles.tile([C, C], f32)
            nc.sync.dma_start(out=g2_sb, in_=g2_d[:, :])

            # bf16 weight casts
            qkvwT_b = singles.tile([C, 3 * C], bf16)
            nc.vector.tensor_copy(out=qkvwT_b, in_=qkvwT_f)
            w4_b = singles.tile([HD, NH, C], bf16)
            nc.vector.tensor_copy(out=w4_b, in_=w4_f)

            # ---- GroupNorm statistics ----
            stats = singles.tile([C, 8, 6], f32)
            for t in range(8):
                nc.vector.bn_stats(
                    out=stats[:, t, :], in_=x_sb[:, t * 512 : (t + 1) * 512]
                )
            mv = singles.tile([C, 2], f32)
            nc.vector.bn_aggr(out=mv, in_=stats)

            # m_ex = [mean_c, E_c[x^2]] per channel
            m_ex = singles.tile([C, 2], f32)
            nc.vector.tensor_copy(out=m_ex[:, 0:1], in_=mv[:, 0:1])
            msq = singles.tile([C, 1], f32)
            nc.vector.tensor_mul(out=msq, in0=mv[:, 0:1], in1=mv[:, 0:1])
            nc.vector.tensor_add(out=m_ex[:, 1:2], in0=mv[:, 1:2], in1=msq)

            xn_b = singles.tile([C, N], bf16)
            k_sb = singles.tile([C, N], bf16)
            q_sb = singles.tile([C, NSLICE], bf16)
            vt_sb = singles.tile([C, MC, NH, 33], bf16)
            nc.vector.memset(vt_sb[:, :, :, 32:33], 1.0)
            outb_eff = singles.tile([C, 1], f32)
            y_out = singles.tile([C, NSLICE], f32)

            # Route g2 through DVE so the (self-loading) fp32 matmul needs a
            # single semaphore wait — walrus allows only one on Matmult.
            g2_v = singles.tile([C, C], f32)
            nc.vector.tensor_copy(out=g2_v, in_=g2_sb)
            vb4_b = singles.tile([HD, NH], bf16)
            nc.vector.tensor_copy(out=vb4_b, in_=vb4_sb)

            with tc.tile_pool(name="ppsum", bufs=2, space="PSUM") as ppool:
                # group-broadcast matmul: per-channel [mu_g, E_g[x^2]]
                gsp = ppool.tile([C, 2], f32, tag="gsp")
                nc.tensor.matmul(out=gsp, lhsT=g2_v, rhs=m_ex, start=True, stop=True)

                mu_g = singles.tile([C, 1], f32)
                nc.vector.tensor_copy(out=mu_g, in_=gsp[:, 0:1])
                musq = singles.tile([C, 1], f32)
                nc.vector.tensor_mul(out=musq, in0=mu_g, in1=mu_g)
                var_g = singles.tile([C, 1], f32)
                nc.vector.tensor_sub(out=var_g, in0=gsp[:, 1:2], in1=musq)

                eps_t = singles.tile([C, 1], f32)
                nc.vector.memset(eps_t, EPS)
                lnv = singles.tile([C, 1], f32)
                nc.scalar.activation(
                    out=lnv, in_=var_g, func=AF.Ln, bias=eps_t, scale=1.0
                )
                rstd = singles.tile([C, 1], f32)
                nc.scalar.activation(out=rstd, in_=lnv, func=AF.Exp, scale=-0.5)

                a_co = singles.tile([C, 1], f32)
                nc.vector.tensor_mul(out=a_co, in0=rstd, in1=gnw_sb)
                tmpb = singles.tile([C, 1], f32)
                nc.vector.tensor_mul(out=tmpb, in0=mu_g, in1=a_co)
                b_co = singles.tile([C, 1], f32)
                nc.vector.tensor_sub(out=b_co, in0=gnb_sb, in1=tmpb)

                # normalized input in bf16: xn = x*A + B
                for t in range(8):
                    nc.vector.tensor_scalar(
                        out=xn_b[:, t * 512 : (t + 1) * 512],
                        in0=x_sb[:, t * 512 : (t + 1) * 512],
                        scalar1=a_co,
                        scalar2=b_co,
                        op0=ALU.mult,
                        op1=ALU.add,
                    )

                # outb_eff = out_b + out_w @ v_bias   (folds v bias into epilogue)
                obe_p = ppool.tile([C, 1], f32, tag="gsp")
                for h in range(NH):
                    nc.tensor.matmul(
                        out=obe_p,
                        lhsT=w4_b[:, h, :],
                        rhs=vb4_b[:, h : h + 1],
                        start=(h == 0),
                        stop=(h == NH - 1),
                    )
                nc.vector.tensor_add(out=outb_eff, in0=obe_p, in1=ob_sb)

                # ---- K / Q projections ----
                for t in range(8):
                    kp = ppool.tile([C, 512], f32, tag="kqp")
                    nc.tensor.matmul(
                        out=kp,
                        lhsT=qkvwT_b[:, C : 2 * C],
                        rhs=xn_b[:, t * 512 : (t + 1) * 512],
                        start=True,
                        stop=True,
                    )
                    nc.vector.tensor_scalar_add(
                        out=k_sb[:, t * 512 : (t + 1) * 512], in0=kp, scalar1=kb_sb
                    )
                for t in range(NT):
                    qp = ppool.tile([C, 512], f32, tag="kqp")
                    nc.tensor.matmul(
                        out=qp,
                        lhsT=qkvwT_b[:, 0:C],
                        rhs=xn_b[:, t * 512 : (t + 1) * 512],
                        start=True,
                        stop=True,
                    )
                    nc.vector.tensor_scalar_add(
                        out=q_sb[:, t * 512 : (t + 1) * 512], in0=qp, scalar1=qb_sb
                    )
                # ---- vT (value transposed, [m, head, d]) via xn-as-lhsT ----
                for mc in range(MC):
                    vp = ppool.tile([C, C], f32, tag="vtp")
                    nc.tensor.matmul(
                        out=vp,
                        lhsT=xn_b[:, mc * 128 : (mc + 1) * 128],
                        rhs=qkvwT_b[:, 2 * C : 3 * C],
                        start=True,
                        stop=True,
                    )
                    nc.scalar.activation(
                        out=vt_sb[:, mc, :, 0:32],
                        in_=vp.rearrange("p (h d) -> p h d", h=NH),
                        func=AF.Copy,
                    )

            # ---- attention ----
            with tc.tile_pool(name="spsum", bufs=1, space="PSUM") as spool, \
                 tc.tile_pool(name="opsum", bufs=1, space="PSUM") as opool, \
                 tc.tile_pool(name="apool", bufs=3) as apool, \
                 tc.tile_pool(name="drpool", bufs=2, space="DRAM") as drpool, \
                 tc.tile_pool(name="dpool", bufs=2) as dpool:
                for nt in range(NT):
                    o_ps = opool.tile([33, NH, 512], f32, tag="oacc")
                    for mc in range(MC):
                        s_ps = spool.tile([C, NH, 512], f32)
                        for h in range(NH):
                            nc.tensor.matmul(
                                out=s_ps[:, h, :],
                                lhsT=k_sb[32 * h : 32 * h + 32, mc * 128 : (mc + 1) * 128],
                                rhs=q_sb[32 * h : 32 * h + 32, nt * 512 : (nt + 1) * 512],
                                start=True,
                                stop=True,
                                tile_position=(32 * h, 0),
                            )
                        e_sb = apool.tile([C, NH, 512], bf16)
                        nc.scalar.activation(out=e_sb, in_=s_ps, func=AF.Exp, scale=SCALE)
                        for h in range(NH):
                            nc.tensor.matmul(
                                out=o_ps[:, h, :],
                                lhsT=vt_sb[:, mc, h, :],
                                rhs=e_sb[:, h, :],
                                start=(mc == 0),
                                stop=(mc == MC - 1),
                            )
                    # ---- per-tile epilogue: 1/Z, out-projection, residual ----
                    r4 = dpool.tile([33, NH, 512], f32)
                    nc.vector.reciprocal(out=r4[32:33, :, :], in_=o_ps[32:33, :, :])
                    r_dram = drpool.tile([NH, 512], f32)
                    nc.sync.dma_start(out=r_dram[:, :], in_=r4[32:33, :, :])
                    rd_ap = r_dram[:, :]
                    r_bcast = bass.AP(
                        tensor=rd_ap.tensor,
                        offset=rd_ap.offset,
                        ap=[[0, 32]] + list(rd_ap.ap),
                    )
                    r_rep = dpool.tile([32, NH, 512], f32)
                    nc.sync.dma_start(out=r_rep, in_=r_bcast)
                    o4 = dpool.tile([32, NH, 512], bf16)
                    nc.vector.tensor_mul(out=o4, in0=o_ps[0:32, :, :], in1=r_rep)

                    y_ps = opool.tile([C, 512], f32, tag="oacc")
                    for h in range(NH):
                        nc.tensor.matmul(
                            out=y_ps,
                            lhsT=w4_b[:, h, :],
                            rhs=o4[:, h, :],
                            start=(h == 0),
                            stop=(h == NH - 1),
                        )
                    nc.vector.scalar_tensor_tensor(
                        out=y_out[:, nt * 512 : (nt + 1) * 512],
                        in0=y_ps,
                        scalar=outb_eff,
                        in1=x_sb[:, nt * 512 : (nt + 1) * 512],
                        op0=ALU.add,
                        op1=ALU.add,
                    )
            nc.sync.dma_start(out=y_d[:, :], in_=y_out)
    nc.finalize()
    return nc


def get_nc():
    if "nc" not in _nc_cache:
        _nc_cache["nc"] = _build()
    return _nc_cache["nc"]


def make_in_maps(inputs):
    x = np.asarray(inputs["x"], dtype=np.float32)
    gn_w = np.asarray(inputs["gn_w"], dtype=np.float32)
    gn_b = np.asarray(inputs["gn_b"], dtype=np.float32)
    qkv_w = np.asarray(inputs["qkv_w"], dtype=np.float32)
    qkv_b = np.asarray(inputs["qkv_b"], dtype=np.float32)
    out_w = np.asarray(inputs["out_w"], dtype=np.float32)
    out_b = np.asarray(inputs["out_b"], dtype=np.float32)

    qkvwT = np.ascontiguousarray(qkv_w.T)                        # [C, 3C]
    # w4[d, h, o] = out_w[o, 32h+d]
    w4 = np.ascontiguousarray(out_w.T.reshape(NH, HD, C).transpose(1, 0, 2))
    qb = np.ascontiguousarray(qkv_b[0:C].reshape(C, 1))
    kb = np.ascontiguousarray(qkv_b[C : 2 * C].reshape(C, 1))
    vb4 = np.ascontiguousarray(qkv_b[2 * C : 3 * C].reshape(NH, HD).T)  # [HD, NH]
    ob = np.ascontiguousarray(out_b.reshape(C, 1))
    gnw = np.ascontiguousarray(gn_w.reshape(C, 1))
    gnb = np.ascontiguousarray(gn_b.reshape(C, 1))
    cidx = np.arange(C)
    g2 = np.where((cidx[:, None] // (C // GROUPS)) == (cidx[None, :] // (C // GROUPS)),
                  np.float32(1.0 / (C // GROUPS)), np.float32(0.0)).astype(np.float32)

    xf = x.reshape(B, C, N)
    in_maps = []
    for core in range(NCORES):
        b, j = divmod(core, NSPLIT)
        n0 = j * NSLICE
        xr = np.ascontiguousarray(np.roll(xf[b], -n0, axis=1))
        in_maps.append(
            {
                "xr": xr,
                "qkvwT": qkvwT,
                "w4": w4,
                "qb": qb,
                "kb": kb,
                "vb4": vb4,
                "ob": ob,
                "gnw": gnw,
                "gnb": gnb,
                "g2": g2,
            }
        )
    return in_maps


def assemble(results):
    y = np.empty((B, C, N), dtype=np.float32)
    for core in range(NCORES):
        b, j = divmod(core, NSPLIT)
        y[b][:, j * NSLICE : (j + 1) * NSLICE] = results[core]["y"]
    return y.reshape(B, C, *D3)


def run(inputs, trace=False):
    from concourse.bass_utils import run_bass_kernel_spmd

    nc = get_nc()
    in_maps = make_in_maps(inputs)
    res = run_bass_kernel_spmd(
        nc, in_maps, core_ids=list(range(NCORES)), trace=trace
    )
    return assemble(res.results), res


def kernel(**inputs):
    out, _ = run(inputs, trace=False)
    return out
